# revision 39
# baseline (speedup 1.0000x reference)
"""HODLR matvec kernel for 8 TRN2 NeuronCores (Bass/Tile).

Sharding: node axis split into 8 contiguous slices of 32768 nodes.
Per core:
  projection  t[l,r,b] = sum_c u[l,c,r] * x[b,c]   (per block, all 8 levels)
              done in two passes: level-pairs (0,1) first, then (2,3),
              so the cross-core collective overlaps the second pass
  tree        combine L7-block partials up to coarser blocks
  A2A         exchange levels 0-2 sibling coefficients across cores
              (sender-side 0/1 masks make the combination core-invariant)
  expansion   corr[b,n] = sum_{l,r} u[l,n,r] * t_sib[l,r,b]
u/x are fed as fp8e4m3 (u scaled by USCALE; host divides the returned
correction by USCALE^2). The expansion runs fp8 DoubleRow matmuls that
contract two level-pairs (K=256) per instruction. Host computes diag*x
in fp32 and adds the device-computed correction.
"""

import os
import sys

sys.path.insert(0, "/opt/trn_rl_repo")

import numpy as np
import ml_dtypes

BF16 = ml_dtypes.bfloat16
FP8 = ml_dtypes.float8_e4m3

B = 64
N = 262144
NCORES = 8
M = N // NCORES          # 32768 nodes per core
R = 64
DEPTH = 8
CH = M // 128            # 256 chunks of 128 nodes
NB7 = M // 1024          # 32 L7 blocks (1024 nodes each)
USCALE = 64.0            # u is fed as u*USCALE in fp8 (e4m3 max finite 240)

_cached = {}


def _build_bass():
    import concourse.bacc as bacc
    import concourse.tile as tile
    import concourse.mybir as mybir
    from contextlib import ExitStack

    BF = mybir.dt.bfloat16
    F8 = mybir.dt.float8e4
    F32 = mybir.dt.float32
    ADD = mybir.AluOpType.add
    MULT = mybir.AluOpType.mult

    nc = bacc.Bacc(
        "TRN2",
        target_bir_lowering=False,
        debug=False,
        enable_asserts=False,
        num_devices=NCORES,
    )

    xt_d = nc.dram_tensor("xt", [128, CH, B], F8, kind="ExternalInput").ap()
    # u split coarse-first: uA = levels 0-2 (192 cols), uB = levels 3-7
    # (320 cols); col order within each is l*64+r
    uA_d = nc.dram_tensor("uA", [128, CH, 192], F8, kind="ExternalInput").ap()
    uB_d = nc.dram_tensor("uB", [128, CH, 320], F8, kind="ExternalInput").ap()
    ut_d = nc.dram_tensor("ut", [4, 128, M], F8, kind="ExternalInput").ap()
    mA_d = nc.dram_tensor("maskA", [128, 8, B], BF, kind="ExternalInput").ap()
    mB_d = nc.dram_tensor("maskB", [64, 8, B], BF, kind="ExternalInput").ap()
    corr_d = nc.dram_tensor("corr", [B, M], F8, kind="ExternalOutput").ap()

    with tile.TileContext(nc) as tc, ExitStack() as ctx:
        const = ctx.enter_context(tc.tile_pool(name="const", bufs=1))
        upool = ctx.enter_context(tc.tile_pool(name="upool", bufs=3))
        y1p = ctx.enter_context(tc.tile_pool(name="y1p", bufs=1))
        pp = ctx.enter_context(tc.tile_pool(name="pp", bufs=2, space="PSUM"))
        accp = ctx.enter_context(tc.tile_pool(name="accp", bufs=1, space="PSUM"))
        accL2 = ctx.enter_context(tc.tile_pool(name="accL2", bufs=1, space="PSUM"))
        accr = ctx.enter_context(tc.tile_pool(name="accr", bufs=2, space="PSUM"))
        treep = ctx.enter_context(tc.tile_pool(name="treep", bufs=1))
        statp = ctx.enter_context(tc.tile_pool(name="statp", bufs=1))
        utp = ctx.enter_context(tc.tile_pool(name="utp", bufs=5))
        ep = ctx.enter_context(tc.tile_pool(name="ep", bufs=2, space="PSUM"))
        yp = ctx.enter_context(tc.tile_pool(name="yp", bufs=2))
        dram = ctx.enter_context(tc.tile_pool(name="dram", bufs=1, space="DRAM"))

        # xt as 4 independent tiles so projection can start after the first
        # quarter lands (whole-tile dependency tracking)
        xt_t = []
        for xq in range(4):
            t_ = const.tile([128, 64, B], F8, tag=f"xt{xq}")
            nc.gpsimd.dma_start(t_[:], xt_d[:, 64 * xq : 64 * (xq + 1), :])
            xt_t.append(t_)

        def xt_chunk(k):
            return xt_t[k // 64][:, k % 64, :]

        mA = const.tile([128, 8, B], BF, tag="mA")
        nc.scalar.dma_start(mA[:], mA_d[:])
        mB = const.tile([64, 8, B], BF, tag="mB")
        nc.scalar.dma_start(mB[:], mB_d[:])

        # ------------- projection with PSUM tree-accumulation -------------
        # Per-level block sums accumulate directly in PSUM across their
        # contributing L7 blocks (start on first MM, stop on last): the DVE
        # tree reduces to a few drains + pair-adds, and the collective's
        # inputs are ready right after the last phase-0 matmul.
        # G[(q, sz)][j]: [128, 64] bf16; rows 0:64 -> level 2q, rows
        # 64:128 -> level 2q+1 of the j-th block of `sz` nodes.
        G = {}

        def tcopy(i, out, in_):
            (nc.vector if i % 2 == 0 else nc.any).tensor_copy(out, in_)

        def tadd(i, out, a, b):
            if i % 2 == 0:
                nc.vector.tensor_tensor(out, a, b, op=ADD)
            else:
                nc.any.tensor_add(out, a, b)

        # long-lived PSUM accumulators: q0 -> one 32768-sum, q1 -> two
        # 16384-halves, q2 -> eight 4096-sums ([128, B] f32 each)
        # hardware clears has_written for the WHOLE 2KB bank on start=True,
        # so each concurrently-open accumulation group gets its own bank
        acc_c = accp.tile([128, 2, B], F32, tag="acc_c", name="acc_c")
        acc_l2 = accL2.tile([64, B], F32, tag="acc_l2", name="acc_l2")

        # ---------------- phase A: levels 0-2 (collective inputs) --------
        for j in range(NB7):
            if j % 4 == 0:
                uA_t = upool.tile(
                    [128, 32, 192], F8, tag="uA", name=f"uA_{j}"
                )
                nc.sync.dma_start(uA_t[:], uA_d[:, 8 * j : 8 * j + 32, :])
            u_t = uA_t[:, 8 * (j % 4) : 8 * (j % 4) + 8, :]
            # 8 consecutive MMs per PSUM target (interleaving targets
            # within a bank serializes the PE pipeline)
            for ki in range(8):
                k = 8 * j + ki
                # L0+L1 -> acc_c[:, 0, :] (the group that starts the bank)
                nc.tensor.matmul(
                    acc_c[:, 0, :],
                    u_t[:, ki, 0:128],
                    xt_chunk(k),
                    start=(j == 0 and ki == 0),
                    stop=(j == NB7 - 1 and ki == 7),
                )
            for ki in range(8):
                k = 8 * j + ki
                # L2 -> its own bank
                nc.tensor.matmul(
                    acc_l2[:],
                    u_t[:, ki, 128:192],
                    xt_chunk(k),
                    start=(j == 0 and ki == 0),
                    stop=(j == NB7 - 1 and ki == 7),
                )

        # drain coarse tops (fp8 halves the collective payload) and launch
        # the collective immediately
        A = treep.tile([128, B], F8, tag="G0_top")
        nc.vector.tensor_copy(A[:], acc_c[:, 0, :])
        Bt = treep.tile([64, B], F8, tag="G1_top")
        nc.vector.tensor_copy(Bt[:], acc_l2[:])
        # ------------- collective (overlaps phase B) -------------
        # AllGather the raw level-0..2 partials; apply the
        # sibling-selection masks on the receive side (the
        # sibling relation is symmetric, so the same masks work).
        b_in = dram.tile([192, B], F8, tag="b_in")
        b_out = dram.tile(
            [8, 192, B], F8, tag="b_out", addr_space="Shared"
        )
        nc.scalar.dma_start(b_in[0:128, :], A[:])
        nc.scalar.dma_start(b_in[128:192, :], Bt[:])
        nc.gpsimd.collective_compute(
            "AllGather",
            mybir.AluOpType.bypass,
            replica_groups=[list(range(NCORES))],
            ins=[b_in.opt()],
            outs=[b_out.opt()],
        )
        # recvs on the SWDGE (gpsimd) queue: they complete only after the
        # AllGather, and on an HWDGE queue their pending completions would
        # serialize every later HWDGE DMA sharing their semaphore lane.
        # Two big AP-transposed DMAs instead of 16 small ones (SWDGE setup
        # is ~1us each).
        recvA = statp.tile([128, 8, B], F8, tag="recvA")
        recvB = statp.tile([64, 8, B], F8, tag="recvB")
        nc.gpsimd.dma_start(
            recvA[:], b_out[:, 0:128, :].transpose([1, 0, 2])
        )
        nc.gpsimd.dma_start(
            recvB[:], b_out[:, 128:192, :].transpose([1, 0, 2])
        )
        # masked receive-combine
        mskA = statp.tile([128, 8, B], BF, tag="mskA")
        mskB = statp.tile([64, 8, B], BF, tag="mskB")
        for k in range(8):
            nc.vector.tensor_tensor(
                mskA[:, k, :], recvA[:, k, :], mA[:, k, :], op=MULT
            )
            nc.vector.tensor_tensor(
                mskB[:, k, :], recvB[:, k, :], mB[:, k, :], op=MULT
            )
        tallA = statp.tile([128, B], BF, tag="tallA")
        tallB = statp.tile([64, B], BF, tag="tallB")
        nc.vector.tensor_tensor(tallA[:], mskA[:, 0, :], mskA[:, 1, :], op=ADD)
        nc.vector.tensor_tensor(tallB[:], mskB[:, 0, :], mskB[:, 1, :], op=ADD)
        for k in range(2, 8):
            nc.vector.tensor_tensor(tallA[:], tallA[:], mskA[:, k, :], op=ADD)
            nc.vector.tensor_tensor(tallB[:], tallB[:], mskB[:, k, :], op=ADD)

        # ---------------- phase B: levels 3-7 ----------------
        acc3_cur = None
        acc2_cur = None
        for j in range(NB7):
            if j % 4 == 0:
                uB_t = upool.tile(
                    [128, 32, 320], F8, tag="uB", name=f"uB_{j}"
                )
                nc.sync.dma_start(uB_t[:], uB_d[:, 8 * j : 8 * j + 32, :])
            u_t = uB_t[:, 8 * (j % 4) : 8 * (j % 4) + 8, :]
            for q in ("L3", 2, 3):
                if q == "L3":
                    if j % 16 == 0:
                        # reuses acc_c's ring slot (lifetimes are sequential)
                        acc3_cur = accp.tile(
                            [128, 2, B], F32, tag="acc_c",
                            name=f"acc3_{j // 16}",
                        )
                    ps = acc3_cur[64:128, 0, :]
                    sl, st, sp = slice(0, 64), j % 16 == 0, j % 16 == 15
                elif q == 2:
                    if j % 4 == 0:
                        acc2_cur = accr.tile(
                            [128, B], F32, tag="accr", name=f"acc2_{j // 4}"
                        )
                    ps = acc2_cur[:]
                    sl, st, sp = slice(64, 192), j % 4 == 0, j % 4 == 3
                else:
                    ps3 = pp.tile([128, B], F32, tag="proj", name=f"ps3_{j}")
                    ps = ps3[:]
                    sl, st, sp = slice(192, 320), True, True
                for ki in range(8):
                    k = 8 * j + ki
                    nc.tensor.matmul(
                        ps,
                        u_t[:, ki, sl],
                        xt_chunk(k),
                        start=(st and ki == 0),
                        stop=(sp and ki == 7),
                    )
                if q == "L3" and j % 16 == 15:
                    g = treep.tile([128, B], BF, tag=f"G1_16384_{j // 16}")
                    nc.vector.tensor_copy(g[64:128, :], acc3_cur[64:128, 0, :])
                    G.setdefault((1, 16384), []).append(g)
                elif q == 2 and j % 4 == 3:
                    g = treep.tile([128, B], BF, tag=f"G2_4096_{j // 4}")
                    tcopy(j // 4, g[:], acc2_cur[:])
                    G.setdefault((2, 4096), []).append(g)
                elif q == 3:
                    g = treep.tile([128, B], BF, tag=f"G3_1024_{j}")
                    tcopy(j, g[:], ps3[:])
                    G.setdefault((3, 1024), []).append(g)

        # pair-add q2's 4096 drains into 8192s, q3's 1024s into 2048s
        g4 = G[(2, 4096)]
        G[(2, 8192)] = []
        for m in range(4):
            g = treep.tile([128, B], BF, tag=f"G2_8192_{m}")
            tadd(m, g[:], g4[2 * m][:], g4[2 * m + 1][:])
            G[(2, 8192)].append(g)
        lst = G[(3, 1024)]
        G[(3, 2048)] = []
        for m in range(16):
            g = treep.tile([128, B], BF, tag=f"G3_2048_{m}")
            tadd(m + 1, g[:], lst[2 * m][:], lst[2 * m + 1][:])
            G[(3, 2048)].append(g)

        # prefetch first expansion ut tiles (sync queue: FIFO behind the u
        # loads, so they stream as soon as u is done -- never blocked behind
        # the collective sends/recvs which live on the scalar queue)
        # ---------------- expansion stationaries (fp8, DoubleRow) --------
        # statf[q-pair-fuse] tiles [128, 2, B]: [:, j, :] holds the
        # stationary of level-pair (2*fuse + j); rows 0:64 = t_sib at the
        # even level of that pair, rows 64:128 = at the odd level.
        statf01 = []
        for m3 in range(2):
            s = statp.tile([128, 2, B], F8, tag=f"sf01_{m3}", name=f"sf01_{m3}")
            nc.vector.tensor_copy(s[:, 0, :], tallA[:])
            nc.vector.tensor_copy(s[0:64, 1, :], tallB[:])
            nc.vector.tensor_copy(
                s[64:128, 1, :], G[(1, 16384)][m3 ^ 1][64:128, :]
            )
            statf01.append(s)
        statf23 = []
        for m7 in range(NB7):
            s = statp.tile([128, 2, B], F8, tag=f"sf23_{m7}", name=f"sf23_{m7}")
            m5 = m7 // 4
            nc.vector.tensor_copy(
                s[0:64, 0, :], G[(2, 8192)][(m5 // 2) ^ 1][0:64, :]
            )
            nc.vector.tensor_copy(
                s[64:128, 0, :], G[(2, 4096)][m5 ^ 1][64:128, :]
            )
            nc.vector.tensor_copy(
                s[0:64, 1, :], G[(3, 2048)][(m7 // 2) ^ 1][0:64, :]
            )
            nc.vector.tensor_copy(
                s[64:128, 1, :], G[(3, 1024)][m7 ^ 1][64:128, :]
            )
            statf23.append(s)

        # ---------------- expansion (DoubleRow fp8, two passes) ----------
        # pass 1 (levels 4-7) has no collective dependency and runs UNDER
        # the AllGather; pass 2 (levels 0-3) adds the coarse contribution
        # once tallA/tallB are in.
        DR = mybir.MatmulPerfMode.DoubleRow
        y1 = []
        for gb in range(8):  # 8 blocks of 8 groups x 512 nodes
            ut1 = utp.tile([128, 2, 4096], F8, tag="utf1", name=f"utf1_{gb}")
            for j in range(2):
                nc.sync.dma_start(
                    ut1[:, j, :], ut_d[2 + j, :, 4096 * gb : 4096 * (gb + 1)]
                )
            yg = y1p.tile([B, 4096], F8, tag=f"y1_{gb}", name=f"y1_{gb}")
            for gg in range(8):
                g = 8 * gb + gg
                eps = ep.tile([B, 512], F32, tag="exp", name=f"e1_{g}")
                sl = slice(512 * gg, 512 * (gg + 1))
                nc.tensor.matmul(
                    eps[:], statf23[g // 2][:], ut1[:, :, sl],
                    start=True, stop=True, perf_mode=DR,
                )
                if gg % 2 == 0:
                    nc.vector.tensor_copy(yg[:, sl], eps[:])
                else:
                    nc.scalar.copy(yg[:, sl], eps[:])
            y1.append(yg)
        for gb in range(8):
            ut0 = utp.tile([128, 2, 4096], F8, tag="utf0", name=f"utf0_{gb}")
            for j in range(2):
                nc.sync.dma_start(
                    ut0[:, j, :], ut_d[j, :, 4096 * gb : 4096 * (gb + 1)]
                )
            y_sb = yp.tile([B, 4096], F8, tag="y")
            for gg in range(8):
                g = 8 * gb + gg
                eps = ep.tile([B, 512], F32, tag="exp", name=f"e0_{g}")
                sl = slice(512 * gg, 512 * (gg + 1))
                nc.tensor.matmul(
                    eps[:], statf01[g // 32][:], ut0[:, :, sl],
                    start=True, stop=True, perf_mode=DR,
                )
                # coarse + fine combine (DVE: 2-input add, PSUM+SBUF)
                nc.vector.tensor_tensor(
                    y_sb[:, sl], eps[:], y1[gb][:, sl], op=ADD
                )
            # corr stores on the gpsimd (SWDGE) queue: keeps the sync queue
            # free to stream ut tiles without head-of-line blocking
            nc.gpsimd.dma_start(
                corr_d[:, 4096 * gb : 4096 * (gb + 1)], y_sb[:]
            )

    nc.compile()
    return nc


def _pack_inputs(x, diag, u):
    """Build per-core input maps. x (B,N,1) f32, u (DEPTH,N,R) f32."""
    in_maps = []
    x2 = np.asarray(x).reshape(B, N)
    u3 = np.asarray(u)
    for c in range(NCORES):
        base = c * M
        xsl = x2[:, base : base + M]                      # (B, M)
        usl = u3[:, base : base + M, :] * USCALE          # (8, M, 64)
        xt = np.ascontiguousarray(
            xsl.T.reshape(CH, 128, B).transpose(1, 0, 2)
        ).astype(FP8)                                     # [128, CH, B]
        upc = (
            usl.transpose(1, 0, 2).reshape(M, 512)        # [n, l*64+r]
            .reshape(CH, 128, 512)
            .transpose(1, 0, 2)
        )                                                 # [128, CH, 512]
        uA = np.ascontiguousarray(upc[:, :, 0:192]).astype(FP8)
        uB = np.ascontiguousarray(upc[:, :, 192:512]).astype(FP8)
        utp = np.ascontiguousarray(
            usl.transpose(0, 2, 1).reshape(4, 128, M)
        ).astype(FP8)                                     # [4, 128, M]
        # masks: mask[d, l] = 1 iff this core c is in the level-l sibling
        # block of destination core d.
        mA = np.zeros((128, 8, B), dtype=BF16)
        mB = np.zeros((64, 8, B), dtype=BF16)
        for d in range(8):
            if (c // 4) == ((d // 4) ^ 1):
                mA[0:64, d, :] = 1.0   # level 0
            if (c // 2) == ((d // 2) ^ 1):
                mA[64:128, d, :] = 1.0  # level 1
            if c == d ^ 1:
                mB[:, d, :] = 1.0       # level 2
        in_maps.append(
            {"xt": xt, "uA": uA, "uB": uB, "ut": utp, "maskA": mA, "maskB": mB}
        )
    return in_maps


last_results = None


def kernel(x, diag, u):
    global last_results
    from concourse.bass_utils import run_bass_kernel_spmd

    if "nc" not in _cached:
        _cached["nc"] = _build_bass()
    nc = _cached["nc"]

    in_maps = _pack_inputs(x, diag, u)
    res = run_bass_kernel_spmd(nc, in_maps, core_ids=list(range(NCORES)))
    last_results = res

    x2 = np.asarray(x, dtype=np.float32).reshape(B, N)
    d2 = np.asarray(diag, dtype=np.float32).reshape(1, N)
    y = d2 * x2
    inv = 1.0 / (USCALE * USCALE)
    for c in range(NCORES):
        y[:, c * M : (c + 1) * M] += res.results[c]["corr"].astype(np.float32) * inv
    return y.reshape(B, N, 1).astype(np.float32)



# revision 40
# speedup vs baseline: 1.2553x; 1.2553x over previous
"""HODLR matvec kernel for 8 TRN2 NeuronCores (Bass/Tile).

Sharding: node axis split into 8 contiguous slices of 32768 nodes.
Per core:
  projection  t[l,r,b] = sum_c u[l,c,r] * x[b,c]   (per block, all 8 levels)
              done in two passes: level-pairs (0,1) first, then (2,3),
              so the cross-core collective overlaps the second pass
  tree        combine L7-block partials up to coarser blocks
  A2A         exchange levels 0-2 sibling coefficients across cores
              (sender-side 0/1 masks make the combination core-invariant)
  expansion   corr[b,n] = sum_{l,r} u[l,n,r] * t_sib[l,r,b]
u/x are fed as fp8e4m3 (u scaled by USCALE; host divides the returned
correction by USCALE^2). The expansion runs fp8 DoubleRow matmuls that
contract two level-pairs (K=256) per instruction. Host computes diag*x
in fp32 and adds the device-computed correction.
"""

import os
import sys

sys.path.insert(0, "/opt/trn_rl_repo")

import numpy as np
import ml_dtypes

BF16 = ml_dtypes.bfloat16
FP8 = ml_dtypes.float8_e4m3

B = 64
N = 262144
NCORES = 8
M = N // NCORES          # 32768 nodes per core
R = 64
DEPTH = 8
CH = M // 128            # 256 chunks of 128 nodes
NB7 = M // 1024          # 32 L7 blocks (1024 nodes each)
USCALE = 64.0            # u is fed as u*USCALE in fp8 (e4m3 max finite 240)

_cached = {}


def _build_bass():
    import concourse.bacc as bacc
    import concourse.tile as tile
    import concourse.mybir as mybir
    from contextlib import ExitStack

    BF = mybir.dt.bfloat16
    F8 = mybir.dt.float8e4
    F32 = mybir.dt.float32
    ADD = mybir.AluOpType.add
    MULT = mybir.AluOpType.mult

    nc = bacc.Bacc(
        "TRN2",
        target_bir_lowering=False,
        debug=False,
        enable_asserts=False,
        num_devices=NCORES,
    )

    xt_d = nc.dram_tensor("xt", [128, CH, B], F8, kind="ExternalInput").ap()
    # u packed phase-major: [:, ph, k, :] holds level-pairs (2ph, 2ph+1)
    u_d = nc.dram_tensor("u", [128, 2, CH, 256], F8, kind="ExternalInput").ap()
    ut_d = nc.dram_tensor("ut", [4, 128, M], F8, kind="ExternalInput").ap()
    mA_d = nc.dram_tensor("maskA", [128, 8, B], BF, kind="ExternalInput").ap()
    mB_d = nc.dram_tensor("maskB", [64, 8, B], BF, kind="ExternalInput").ap()
    corr_d = nc.dram_tensor("corr", [B, M], F8, kind="ExternalOutput").ap()

    with tile.TileContext(nc) as tc, ExitStack() as ctx:
        const = ctx.enter_context(tc.tile_pool(name="const", bufs=1))
        upool = ctx.enter_context(tc.tile_pool(name="upool", bufs=7))
        pp = ctx.enter_context(tc.tile_pool(name="pp", bufs=2, space="PSUM"))
        accp = ctx.enter_context(tc.tile_pool(name="accp", bufs=1, space="PSUM"))
        accr = ctx.enter_context(tc.tile_pool(name="accr", bufs=2, space="PSUM"))
        treep = ctx.enter_context(tc.tile_pool(name="treep", bufs=1))
        statp = ctx.enter_context(tc.tile_pool(name="statp", bufs=1))
        utp = ctx.enter_context(tc.tile_pool(name="utp", bufs=5))
        ep = ctx.enter_context(tc.tile_pool(name="ep", bufs=3, space="PSUM"))
        yp = ctx.enter_context(tc.tile_pool(name="yp", bufs=3))
        dram = ctx.enter_context(tc.tile_pool(name="dram", bufs=1, space="DRAM"))

        # xt as 4 independent tiles so projection can start after the first
        # quarter lands (whole-tile dependency tracking)
        xt_t = []
        for xq in range(4):
            t_ = const.tile([128, 64, B], F8, tag=f"xt{xq}")
            nc.gpsimd.dma_start(t_[:], xt_d[:, 64 * xq : 64 * (xq + 1), :])
            xt_t.append(t_)

        def xt_chunk(k):
            return xt_t[k // 64][:, k % 64, :]

        mA = const.tile([128, 8, B], BF, tag="mA")
        nc.scalar.dma_start(mA[:], mA_d[:])
        mB = const.tile([64, 8, B], BF, tag="mB")
        nc.scalar.dma_start(mB[:], mB_d[:])

        # ------------- projection with PSUM tree-accumulation -------------
        # Per-level block sums accumulate directly in PSUM across their
        # contributing L7 blocks (start on first MM, stop on last): the DVE
        # tree reduces to a few drains + pair-adds, and the collective's
        # inputs are ready right after the last phase-0 matmul.
        # G[(q, sz)][j]: [128, 64] bf16; rows 0:64 -> level 2q, rows
        # 64:128 -> level 2q+1 of the j-th block of `sz` nodes.
        G = {}

        def tcopy(i, out, in_):
            (nc.vector if i % 2 == 0 else nc.any).tensor_copy(out, in_)

        def tadd(i, out, a, b):
            if i % 2 == 0:
                nc.vector.tensor_tensor(out, a, b, op=ADD)
            else:
                nc.any.tensor_add(out, a, b)

        # long-lived PSUM accumulators: q0 -> one 32768-sum, q1 -> two
        # 16384-halves, q2 -> eight 4096-sums ([128, B] f32 each)
        # hardware clears has_written for the WHOLE 2KB bank on start=True,
        # so every concurrently-open accumulation group gets its own bank:
        # acc0 holds one bank all of phase 0; the q1-half / q2-4096 sums
        # cycle through a 2-bank ring, drained eagerly at their stop.
        acc0 = accp.tile([128, B], F32, tag="acc0", name="acc0")
        acc_cur = None

        for ph, qs in ((0, (0, 1)), (1, (2, 3))):
            for j in range(NB7):
                if j % 4 == 0:
                    u_t2 = upool.tile(
                        [128, 32, 256], F8, tag="u_in", name=f"u_t{ph}_{j}"
                    )
                    nc.sync.dma_start(
                        u_t2[:], u_d[:, ph, 8 * j : 8 * j + 32, :]
                    )
                u_t = u_t2[:, 8 * (j % 4) : 8 * (j % 4) + 8, :]
                for qi, q in enumerate(qs):
                    if q == 0:
                        ps, st, sp = acc0, j == 0, j == NB7 - 1
                    elif q == 1:
                        if j % 16 == 0:
                            acc_cur = accr.tile(
                                [128, B], F32, tag="accr",
                                name=f"acc1_{j // 16}",
                            )
                        ps, st, sp = acc_cur, j % 16 == 0, j % 16 == 15
                    elif q == 2:
                        if j % 4 == 0:
                            acc_cur = accr.tile(
                                [128, B], F32, tag="accr",
                                name=f"acc2_{j // 4}",
                            )
                        ps, st, sp = acc_cur, j % 4 == 0, j % 4 == 3
                    else:
                        ps = pp.tile([128, B], F32, tag="proj", name=f"ps3_{j}")
                        st, sp = True, True
                    for ki in range(8):
                        k = 8 * j + ki
                        nc.tensor.matmul(
                            ps[:],
                            u_t[:, ki, 128 * qi : 128 * qi + 128],
                            xt_chunk(k),
                            start=(st and ki == 0),
                            stop=(sp and ki == 7),
                        )
                    if q == 1 and j % 16 == 15:
                        g = treep.tile([128, B], BF, tag=f"G1_16384_{j // 16}")
                        nc.vector.tensor_copy(g[:], ps[:])
                        G.setdefault((1, 16384), []).append(g)
                    elif q == 2 and j % 4 == 3:
                        g = treep.tile([128, B], BF, tag=f"G2_4096_{j // 4}")
                        tcopy(j // 4, g[:], ps[:])
                        G.setdefault((2, 4096), []).append(g)
                    elif q == 3:
                        g = treep.tile([128, B], BF, tag=f"G3_1024_{j}")
                        tcopy(j, g[:], ps[:])
                        G.setdefault((3, 1024), []).append(g)

            if ph == 0:
                # drain the q0 top and launch the collective immediately
                A = treep.tile([128, B], BF, tag="G0_top")
                nc.vector.tensor_copy(A[:], acc0[:])
                g1h = G[(1, 16384)]
                Bt = treep.tile([64, B], BF, tag="G1_top")
                nc.vector.tensor_tensor(
                    Bt[:], g1h[0][0:64, :], g1h[1][0:64, :], op=ADD
                )
                # ------------- collective (overlaps phase 1) -------------
                # AllGather the raw level-0..2 partials; apply the
                # sibling-selection masks on the receive side (the
                # sibling relation is symmetric, so the same masks work).
                b_in = dram.tile([192, B], BF, tag="b_in")
                b_out = dram.tile(
                    [8, 192, B], BF, tag="b_out", addr_space="Shared"
                )
                nc.scalar.dma_start(b_in[0:128, :], A[:])
                nc.scalar.dma_start(b_in[128:192, :], Bt[:])
                nc.gpsimd.collective_compute(
                    "AllGather",
                    mybir.AluOpType.bypass,
                    replica_groups=[list(range(NCORES))],
                    ins=[b_in.opt()],
                    outs=[b_out.opt()],
                )
                recvA = statp.tile([128, 8, B], BF, tag="recvA")
                recvB = statp.tile([64, 8, B], BF, tag="recvB")
                for k in range(8):
                    nc.scalar.dma_start(recvA[:, k, :], b_out[k, 0:128, :])
                    nc.scalar.dma_start(recvB[:, k, :], b_out[k, 128:192, :])
                # masked receive-combine
                mskA = statp.tile([128, 8, B], BF, tag="mskA")
                mskB = statp.tile([64, 8, B], BF, tag="mskB")
                for k in range(8):
                    nc.vector.tensor_tensor(
                        mskA[:, k, :], recvA[:, k, :], mA[:, k, :], op=MULT
                    )
                    nc.vector.tensor_tensor(
                        mskB[:, k, :], recvB[:, k, :], mB[:, k, :], op=MULT
                    )
                tallA = statp.tile([128, B], BF, tag="tallA")
                tallB = statp.tile([64, B], BF, tag="tallB")
                nc.vector.tensor_tensor(
                    tallA[:], mskA[:, 0, :], mskA[:, 1, :], op=ADD
                )
                nc.vector.tensor_tensor(
                    tallB[:], mskB[:, 0, :], mskB[:, 1, :], op=ADD
                )
                for k in range(2, 8):
                    nc.vector.tensor_tensor(
                        tallA[:], tallA[:], mskA[:, k, :], op=ADD
                    )
                    nc.vector.tensor_tensor(
                        tallB[:], tallB[:], mskB[:, k, :], op=ADD
                    )
            else:
                # pair-add q2's 4096 drains into 8192s, q3's 1024s into 2048s
                g4 = G[(2, 4096)]
                G[(2, 8192)] = []
                for m in range(4):
                    g = treep.tile([128, B], BF, tag=f"G2_8192_{m}")
                    tadd(m, g[:], g4[2 * m][:], g4[2 * m + 1][:])
                    G[(2, 8192)].append(g)
                lst = G[(3, 1024)]
                G[(3, 2048)] = []
                for m in range(16):
                    g = treep.tile([128, B], BF, tag=f"G3_2048_{m}")
                    tadd(m + 1, g[:], lst[2 * m][:], lst[2 * m + 1][:])
                    G[(3, 2048)].append(g)

        # prefetch first expansion ut tiles (sync queue: FIFO behind the u
        # loads, so they stream as soon as u is done -- never blocked behind
        # the collective sends/recvs which live on the scalar queue)
        ut_pre = []
        for f in range(2):
            t_ = utp.tile([128, 2, 4096], F8, tag=f"utf{f}", name=f"ut_pre{f}")
            for j in range(2):
                nc.sync.dma_start(t_[:, j, :], ut_d[2 * f + j, :, 0:4096])
            ut_pre.append(t_)

        # ---------------- expansion stationaries (fp8, DoubleRow) --------
        # statf[q-pair-fuse] tiles [128, 2, B]: [:, j, :] holds the
        # stationary of level-pair (2*fuse + j); rows 0:64 = t_sib at the
        # even level of that pair, rows 64:128 = at the odd level.
        statf01 = []
        for m3 in range(2):
            s = statp.tile([128, 2, B], F8, tag=f"sf01_{m3}", name=f"sf01_{m3}")
            nc.vector.tensor_copy(s[:, 0, :], tallA[:])
            nc.vector.tensor_copy(s[0:64, 1, :], tallB[:])
            nc.vector.tensor_copy(
                s[64:128, 1, :], G[(1, 16384)][m3 ^ 1][64:128, :]
            )
            statf01.append(s)
        statf23 = []
        for m7 in range(NB7):
            s = statp.tile([128, 2, B], F8, tag=f"sf23_{m7}", name=f"sf23_{m7}")
            m5 = m7 // 4
            nc.vector.tensor_copy(
                s[0:64, 0, :], G[(2, 8192)][(m5 // 2) ^ 1][0:64, :]
            )
            nc.vector.tensor_copy(
                s[64:128, 0, :], G[(2, 4096)][m5 ^ 1][64:128, :]
            )
            nc.vector.tensor_copy(
                s[0:64, 1, :], G[(3, 2048)][(m7 // 2) ^ 1][0:64, :]
            )
            nc.vector.tensor_copy(
                s[64:128, 1, :], G[(3, 1024)][m7 ^ 1][64:128, :]
            )
            statf23.append(s)

        # ---------------- expansion (DoubleRow fp8) ----------------
        DR = mybir.MatmulPerfMode.DoubleRow
        for gb in range(8):  # 8 blocks of 8 groups x 512 nodes
            if gb == 0:
                ut_t = ut_pre
            else:
                ut_t = [
                    utp.tile(
                        [128, 2, 4096], F8, tag=f"utf{f}", name=f"utf{f}_{gb}"
                    )
                    for f in range(2)
                ]
                for f in range(2):
                    for j in range(2):
                        nc.sync.dma_start(
                            ut_t[f][:, j, :],
                            ut_d[2 * f + j, :, 4096 * gb : 4096 * (gb + 1)],
                        )
            y_sb = yp.tile([B, 4096], F8, tag="y")
            for gg in range(8):
                g = 8 * gb + gg
                eps = ep.tile([B, 512], F32, tag="exp", name=f"eps{g}")
                sl = slice(512 * gg, 512 * (gg + 1))
                nc.tensor.matmul(
                    eps[:], statf23[g // 2][:], ut_t[1][:, :, sl],
                    start=True, stop=False, perf_mode=DR,
                )
                nc.tensor.matmul(
                    eps[:], statf01[g // 32][:], ut_t[0][:, :, sl],
                    start=False, stop=True, perf_mode=DR,
                )
                # split PSUM drains across DVE and ACT (both can read PSUM)
                if gg % 2 == 0:
                    nc.vector.tensor_copy(y_sb[:, sl], eps[:])
                else:
                    nc.scalar.copy(y_sb[:, sl], eps[:])
            # corr stores on the gpsimd (SWDGE) queue: keeps the sync queue
            # free to stream ut tiles without head-of-line blocking
            nc.gpsimd.dma_start(
                corr_d[:, 4096 * gb : 4096 * (gb + 1)], y_sb[:]
            )

    nc.compile()
    return nc


def _pack_inputs(x, diag, u):
    """Build per-core input maps. x (B,N,1) f32, u (DEPTH,N,R) f32."""
    in_maps = []
    x2 = np.asarray(x).reshape(B, N)
    u3 = np.asarray(u)
    for c in range(NCORES):
        base = c * M
        xsl = x2[:, base : base + M]                      # (B, M)
        usl = u3[:, base : base + M, :] * USCALE          # (8, M, 64)
        xt = np.ascontiguousarray(
            xsl.T.reshape(CH, 128, B).transpose(1, 0, 2)
        ).astype(FP8)                                     # [128, CH, B]
        up = np.ascontiguousarray(
            usl.transpose(1, 0, 2).reshape(M, 512)        # [n, l*64+r]
            .reshape(CH, 128, 2, 256)
            .transpose(1, 2, 0, 3)
        ).astype(FP8)                                     # [128, 2, CH, 256]
        utp = np.ascontiguousarray(
            usl.transpose(0, 2, 1).reshape(4, 128, M)
        ).astype(FP8)                                     # [4, 128, M]
        # masks: mask[d, l] = 1 iff this core c is in the level-l sibling
        # block of destination core d.
        mA = np.zeros((128, 8, B), dtype=BF16)
        mB = np.zeros((64, 8, B), dtype=BF16)
        for d in range(8):
            if (c // 4) == ((d // 4) ^ 1):
                mA[0:64, d, :] = 1.0   # level 0
            if (c // 2) == ((d // 2) ^ 1):
                mA[64:128, d, :] = 1.0  # level 1
            if c == d ^ 1:
                mB[:, d, :] = 1.0       # level 2
        in_maps.append({"xt": xt, "u": up, "ut": utp, "maskA": mA, "maskB": mB})
    return in_maps


last_results = None


def kernel(x, diag, u):
    global last_results
    from concourse.bass_utils import run_bass_kernel_spmd

    if "nc" not in _cached:
        _cached["nc"] = _build_bass()
    nc = _cached["nc"]

    in_maps = _pack_inputs(x, diag, u)
    res = run_bass_kernel_spmd(nc, in_maps, core_ids=list(range(NCORES)))
    last_results = res

    x2 = np.asarray(x, dtype=np.float32).reshape(B, N)
    d2 = np.asarray(diag, dtype=np.float32).reshape(1, N)
    y = d2 * x2
    inv = 1.0 / (USCALE * USCALE)
    for c in range(NCORES):
        y[:, c * M : (c + 1) * M] += res.results[c]["corr"].astype(np.float32) * inv
    return y.reshape(B, N, 1).astype(np.float32)



# revision 41
# speedup vs baseline: 1.2712x; 1.0127x over previous
"""HODLR matvec kernel for 8 TRN2 NeuronCores (Bass/Tile).

Sharding: node axis split into 8 contiguous slices of 32768 nodes.
Per core:
  projection  t[l,r,b] = sum_c u[l,c,r] * x[b,c]   (per block, all 8 levels)
              done in two passes: level-pairs (0,1) first, then (2,3),
              so the cross-core collective overlaps the second pass
  tree        combine L7-block partials up to coarser blocks
  A2A         exchange levels 0-2 sibling coefficients across cores
              (sender-side 0/1 masks make the combination core-invariant)
  expansion   corr[b,n] = sum_{l,r} u[l,n,r] * t_sib[l,r,b]
u/x are fed as fp8e4m3 (u scaled by USCALE; host divides the returned
correction by USCALE^2). The expansion runs fp8 DoubleRow matmuls that
contract two level-pairs (K=256) per instruction. Host computes diag*x
in fp32 and adds the device-computed correction.
"""

import os
import sys

sys.path.insert(0, "/opt/trn_rl_repo")

import numpy as np
import ml_dtypes

BF16 = ml_dtypes.bfloat16
FP8 = ml_dtypes.float8_e4m3

B = 64
N = 262144
NCORES = 8
M = N // NCORES          # 32768 nodes per core
R = 64
DEPTH = 8
CH = M // 128            # 256 chunks of 128 nodes
NB7 = M // 1024          # 32 L7 blocks (1024 nodes each)
USCALE = 64.0            # u is fed as u*USCALE in fp8 (e4m3 max finite 240)

_cached = {}


def _build_bass():
    import concourse.bacc as bacc
    import concourse.tile as tile
    import concourse.mybir as mybir
    from contextlib import ExitStack

    BF = mybir.dt.bfloat16
    F8 = mybir.dt.float8e4
    F32 = mybir.dt.float32
    ADD = mybir.AluOpType.add
    MULT = mybir.AluOpType.mult

    nc = bacc.Bacc(
        "TRN2",
        target_bir_lowering=False,
        debug=False,
        enable_asserts=False,
        num_devices=NCORES,
    )

    xt_d = nc.dram_tensor("xt", [128, CH, B], F8, kind="ExternalInput").ap()
    # u packed phase-major: [:, ph, k, :] holds level-pairs (2ph, 2ph+1)
    u_d = nc.dram_tensor("u", [128, 2, CH, 256], F8, kind="ExternalInput").ap()
    ut_d = nc.dram_tensor("ut", [4, 128, M], F8, kind="ExternalInput").ap()
    mA_d = nc.dram_tensor("maskA", [128, 8, B], BF, kind="ExternalInput").ap()
    mB_d = nc.dram_tensor("maskB", [64, 8, B], BF, kind="ExternalInput").ap()
    corr_d = nc.dram_tensor("corr", [B, M], F8, kind="ExternalOutput").ap()

    with tile.TileContext(nc) as tc, ExitStack() as ctx:
        const = ctx.enter_context(tc.tile_pool(name="const", bufs=1))
        upool = ctx.enter_context(tc.tile_pool(name="upool", bufs=7))
        pp = ctx.enter_context(tc.tile_pool(name="pp", bufs=2, space="PSUM"))
        accp = ctx.enter_context(tc.tile_pool(name="accp", bufs=1, space="PSUM"))
        accr = ctx.enter_context(tc.tile_pool(name="accr", bufs=2, space="PSUM"))
        treep = ctx.enter_context(tc.tile_pool(name="treep", bufs=1))
        statp = ctx.enter_context(tc.tile_pool(name="statp", bufs=1))
        utp = ctx.enter_context(tc.tile_pool(name="utp", bufs=5))
        ep = ctx.enter_context(tc.tile_pool(name="ep", bufs=3, space="PSUM"))
        yp = ctx.enter_context(tc.tile_pool(name="yp", bufs=3))
        dram = ctx.enter_context(tc.tile_pool(name="dram", bufs=1, space="DRAM"))

        # xt as 4 independent tiles so projection can start after the first
        # quarter lands (whole-tile dependency tracking)
        xt_t = []
        for xq in range(4):
            t_ = const.tile([128, 64, B], F8, tag=f"xt{xq}")
            nc.gpsimd.dma_start(t_[:], xt_d[:, 64 * xq : 64 * (xq + 1), :])
            xt_t.append(t_)

        def xt_chunk(k):
            return xt_t[k // 64][:, k % 64, :]

        mA = const.tile([128, 8, B], BF, tag="mA")
        nc.scalar.dma_start(mA[:], mA_d[:])
        mB = const.tile([64, 8, B], BF, tag="mB")
        nc.scalar.dma_start(mB[:], mB_d[:])

        # ------------- projection with PSUM tree-accumulation -------------
        # Per-level block sums accumulate directly in PSUM across their
        # contributing L7 blocks (start on first MM, stop on last): the DVE
        # tree reduces to a few drains + pair-adds, and the collective's
        # inputs are ready right after the last phase-0 matmul.
        # G[(q, sz)][j]: [128, 64] bf16; rows 0:64 -> level 2q, rows
        # 64:128 -> level 2q+1 of the j-th block of `sz` nodes.
        G = {}

        def tcopy(i, out, in_):
            (nc.vector if i % 2 == 0 else nc.any).tensor_copy(out, in_)

        def tadd(i, out, a, b):
            if i % 2 == 0:
                nc.vector.tensor_tensor(out, a, b, op=ADD)
            else:
                nc.any.tensor_add(out, a, b)

        # long-lived PSUM accumulators: q0 -> one 32768-sum, q1 -> two
        # 16384-halves, q2 -> eight 4096-sums ([128, B] f32 each)
        # hardware clears has_written for the WHOLE 2KB bank on start=True,
        # so every concurrently-open accumulation group gets its own bank:
        # acc0 holds one bank all of phase 0; the q1-half / q2-4096 sums
        # cycle through a 2-bank ring, drained eagerly at their stop.
        acc0 = accp.tile([128, B], F32, tag="acc0", name="acc0")
        acc_cur = None

        for ph, qs in ((0, (0, 1)), (1, (2, 3))):
            for j in range(NB7):
                if j % 4 == 0:
                    u_t2 = upool.tile(
                        [128, 32, 256], F8, tag="u_in", name=f"u_t{ph}_{j}"
                    )
                    nc.sync.dma_start(
                        u_t2[:], u_d[:, ph, 8 * j : 8 * j + 32, :]
                    )
                u_t = u_t2[:, 8 * (j % 4) : 8 * (j % 4) + 8, :]
                for qi, q in enumerate(qs):
                    if q == 0:
                        ps, st, sp = acc0, j == 0, j == NB7 - 1
                    elif q == 1:
                        if j % 16 == 0:
                            acc_cur = accr.tile(
                                [128, B], F32, tag="accr",
                                name=f"acc1_{j // 16}",
                            )
                        ps, st, sp = acc_cur, j % 16 == 0, j % 16 == 15
                    elif q == 2:
                        if j % 4 == 0:
                            acc_cur = accr.tile(
                                [128, B], F32, tag="accr",
                                name=f"acc2_{j // 4}",
                            )
                        ps, st, sp = acc_cur, j % 4 == 0, j % 4 == 3
                    else:
                        ps = pp.tile([128, B], F32, tag="proj", name=f"ps3_{j}")
                        st, sp = True, True
                    for ki in range(8):
                        k = 8 * j + ki
                        nc.tensor.matmul(
                            ps[:],
                            u_t[:, ki, 128 * qi : 128 * qi + 128],
                            xt_chunk(k),
                            start=(st and ki == 0),
                            stop=(sp and ki == 7),
                        )
                    if q == 1 and j % 16 == 15:
                        g = treep.tile([128, B], BF, tag=f"G1_16384_{j // 16}")
                        nc.vector.tensor_copy(g[:], ps[:])
                        G.setdefault((1, 16384), []).append(g)
                    elif q == 2 and j % 4 == 3:
                        g = treep.tile([128, B], BF, tag=f"G2_4096_{j // 4}")
                        tcopy(j // 4, g[:], ps[:])
                        G.setdefault((2, 4096), []).append(g)
                    elif q == 3:
                        g = treep.tile([128, B], BF, tag=f"G3_1024_{j}")
                        tcopy(j, g[:], ps[:])
                        G.setdefault((3, 1024), []).append(g)

            if ph == 0:
                # drain the q0 top and launch the collective immediately
                A = treep.tile([128, B], BF, tag="G0_top")
                nc.vector.tensor_copy(A[:], acc0[:])
                g1h = G[(1, 16384)]
                Bt = treep.tile([64, B], BF, tag="G1_top")
                nc.vector.tensor_tensor(
                    Bt[:], g1h[0][0:64, :], g1h[1][0:64, :], op=ADD
                )
                # ------------- collective (overlaps phase 1) -------------
                # AllGather the raw level-0..2 partials; apply the
                # sibling-selection masks on the receive side (the
                # sibling relation is symmetric, so the same masks work).
                b_in = dram.tile([192, B], BF, tag="b_in")
                b_out = dram.tile(
                    [8, 192, B], BF, tag="b_out", addr_space="Shared"
                )
                nc.scalar.dma_start(b_in[0:128, :], A[:])
                nc.scalar.dma_start(b_in[128:192, :], Bt[:])
                nc.gpsimd.collective_compute(
                    "AllGather",
                    mybir.AluOpType.bypass,
                    replica_groups=[list(range(NCORES))],
                    ins=[b_in.opt()],
                    outs=[b_out.opt()],
                )
                # recvs as two batched SWDGE (gpsimd) DMAs: they complete
                # only after the AllGather, and on an HWDGE queue their
                # pending completions would head-of-line block every later
                # HWDGE DMA sharing their semaphore lane (the ut prefetch)
                recvA = statp.tile([128, 8, B], BF, tag="recvA")
                recvB = statp.tile([64, 8, B], BF, tag="recvB")
                nc.gpsimd.dma_start(
                    recvA[:], b_out[:, 0:128, :].transpose([1, 0, 2])
                )
                nc.gpsimd.dma_start(
                    recvB[:], b_out[:, 128:192, :].transpose([1, 0, 2])
                )
                # masked receive-combine
                mskA = statp.tile([128, 8, B], BF, tag="mskA")
                mskB = statp.tile([64, 8, B], BF, tag="mskB")
                for k in range(8):
                    nc.vector.tensor_tensor(
                        mskA[:, k, :], recvA[:, k, :], mA[:, k, :], op=MULT
                    )
                    nc.vector.tensor_tensor(
                        mskB[:, k, :], recvB[:, k, :], mB[:, k, :], op=MULT
                    )
                tallA = statp.tile([128, B], BF, tag="tallA")
                tallB = statp.tile([64, B], BF, tag="tallB")
                nc.vector.tensor_tensor(
                    tallA[:], mskA[:, 0, :], mskA[:, 1, :], op=ADD
                )
                nc.vector.tensor_tensor(
                    tallB[:], mskB[:, 0, :], mskB[:, 1, :], op=ADD
                )
                for k in range(2, 8):
                    nc.vector.tensor_tensor(
                        tallA[:], tallA[:], mskA[:, k, :], op=ADD
                    )
                    nc.vector.tensor_tensor(
                        tallB[:], tallB[:], mskB[:, k, :], op=ADD
                    )
            else:
                # pair-add q2's 4096 drains into 8192s, q3's 1024s into 2048s
                g4 = G[(2, 4096)]
                G[(2, 8192)] = []
                for m in range(4):
                    g = treep.tile([128, B], BF, tag=f"G2_8192_{m}")
                    tadd(m, g[:], g4[2 * m][:], g4[2 * m + 1][:])
                    G[(2, 8192)].append(g)
                lst = G[(3, 1024)]
                G[(3, 2048)] = []
                for m in range(16):
                    g = treep.tile([128, B], BF, tag=f"G3_2048_{m}")
                    tadd(m + 1, g[:], lst[2 * m][:], lst[2 * m + 1][:])
                    G[(3, 2048)].append(g)

        # prefetch first expansion ut tiles (sync queue: FIFO behind the u
        # loads, so they stream as soon as u is done -- never blocked behind
        # the collective sends/recvs which live on the scalar queue)
        ut_pre = []
        for f in range(2):
            t_ = utp.tile([128, 2, 4096], F8, tag=f"utf{f}", name=f"ut_pre{f}")
            for j in range(2):
                nc.sync.dma_start(t_[:, j, :], ut_d[2 * f + j, :, 0:4096])
            ut_pre.append(t_)

        # ---------------- expansion stationaries (fp8, DoubleRow) --------
        # statf[q-pair-fuse] tiles [128, 2, B]: [:, j, :] holds the
        # stationary of level-pair (2*fuse + j); rows 0:64 = t_sib at the
        # even level of that pair, rows 64:128 = at the odd level.
        statf01 = []
        for m3 in range(2):
            s = statp.tile([128, 2, B], F8, tag=f"sf01_{m3}", name=f"sf01_{m3}")
            nc.vector.tensor_copy(s[:, 0, :], tallA[:])
            nc.vector.tensor_copy(s[0:64, 1, :], tallB[:])
            nc.vector.tensor_copy(
                s[64:128, 1, :], G[(1, 16384)][m3 ^ 1][64:128, :]
            )
            statf01.append(s)
        statf23 = []
        for m7 in range(NB7):
            s = statp.tile([128, 2, B], F8, tag=f"sf23_{m7}", name=f"sf23_{m7}")
            m5 = m7 // 4
            nc.vector.tensor_copy(
                s[0:64, 0, :], G[(2, 8192)][(m5 // 2) ^ 1][0:64, :]
            )
            nc.vector.tensor_copy(
                s[64:128, 0, :], G[(2, 4096)][m5 ^ 1][64:128, :]
            )
            nc.vector.tensor_copy(
                s[0:64, 1, :], G[(3, 2048)][(m7 // 2) ^ 1][0:64, :]
            )
            nc.vector.tensor_copy(
                s[64:128, 1, :], G[(3, 1024)][m7 ^ 1][64:128, :]
            )
            statf23.append(s)

        # ---------------- expansion (DoubleRow fp8) ----------------
        DR = mybir.MatmulPerfMode.DoubleRow
        for gb in range(8):  # 8 blocks of 8 groups x 512 nodes
            if gb == 0:
                ut_t = ut_pre
            else:
                ut_t = [
                    utp.tile(
                        [128, 2, 4096], F8, tag=f"utf{f}", name=f"utf{f}_{gb}"
                    )
                    for f in range(2)
                ]
                for f in range(2):
                    for j in range(2):
                        nc.sync.dma_start(
                            ut_t[f][:, j, :],
                            ut_d[2 * f + j, :, 4096 * gb : 4096 * (gb + 1)],
                        )
            y_sb = yp.tile([B, 4096], F8, tag="y")
            for gg in range(8):
                g = 8 * gb + gg
                eps = ep.tile([B, 512], F32, tag="exp", name=f"eps{g}")
                sl = slice(512 * gg, 512 * (gg + 1))
                nc.tensor.matmul(
                    eps[:], statf23[g // 2][:], ut_t[1][:, :, sl],
                    start=True, stop=False, perf_mode=DR,
                )
                nc.tensor.matmul(
                    eps[:], statf01[g // 32][:], ut_t[0][:, :, sl],
                    start=False, stop=True, perf_mode=DR,
                )
                # split PSUM drains across DVE and ACT (both can read PSUM)
                if gg % 2 == 0:
                    nc.vector.tensor_copy(y_sb[:, sl], eps[:])
                else:
                    nc.scalar.copy(y_sb[:, sl], eps[:])
            # corr stores on the gpsimd (SWDGE) queue: keeps the sync queue
            # free to stream ut tiles without head-of-line blocking
            nc.gpsimd.dma_start(
                corr_d[:, 4096 * gb : 4096 * (gb + 1)], y_sb[:]
            )

    nc.compile()
    return nc


def _pack_inputs(x, diag, u):
    """Build per-core input maps. x (B,N,1) f32, u (DEPTH,N,R) f32."""
    in_maps = []
    x2 = np.asarray(x).reshape(B, N)
    u3 = np.asarray(u)
    for c in range(NCORES):
        base = c * M
        xsl = x2[:, base : base + M]                      # (B, M)
        usl = u3[:, base : base + M, :] * USCALE          # (8, M, 64)
        xt = np.ascontiguousarray(
            xsl.T.reshape(CH, 128, B).transpose(1, 0, 2)
        ).astype(FP8)                                     # [128, CH, B]
        up = np.ascontiguousarray(
            usl.transpose(1, 0, 2).reshape(M, 512)        # [n, l*64+r]
            .reshape(CH, 128, 2, 256)
            .transpose(1, 2, 0, 3)
        ).astype(FP8)                                     # [128, 2, CH, 256]
        utp = np.ascontiguousarray(
            usl.transpose(0, 2, 1).reshape(4, 128, M)
        ).astype(FP8)                                     # [4, 128, M]
        # masks: mask[d, l] = 1 iff this core c is in the level-l sibling
        # block of destination core d.
        mA = np.zeros((128, 8, B), dtype=BF16)
        mB = np.zeros((64, 8, B), dtype=BF16)
        for d in range(8):
            if (c // 4) == ((d // 4) ^ 1):
                mA[0:64, d, :] = 1.0   # level 0
            if (c // 2) == ((d // 2) ^ 1):
                mA[64:128, d, :] = 1.0  # level 1
            if c == d ^ 1:
                mB[:, d, :] = 1.0       # level 2
        in_maps.append({"xt": xt, "u": up, "ut": utp, "maskA": mA, "maskB": mB})
    return in_maps


last_results = None


def kernel(x, diag, u):
    global last_results
    from concourse.bass_utils import run_bass_kernel_spmd

    if "nc" not in _cached:
        _cached["nc"] = _build_bass()
    nc = _cached["nc"]

    in_maps = _pack_inputs(x, diag, u)
    res = run_bass_kernel_spmd(nc, in_maps, core_ids=list(range(NCORES)))
    last_results = res

    x2 = np.asarray(x, dtype=np.float32).reshape(B, N)
    d2 = np.asarray(diag, dtype=np.float32).reshape(1, N)
    y = d2 * x2
    inv = 1.0 / (USCALE * USCALE)
    for c in range(NCORES):
        y[:, c * M : (c + 1) * M] += res.results[c]["corr"].astype(np.float32) * inv
    return y.reshape(B, N, 1).astype(np.float32)



# revision 44
# speedup vs baseline: 1.2785x; 1.0057x over previous
"""HODLR matvec kernel for 8 TRN2 NeuronCores (Bass/Tile).

Sharding: node axis split into 8 contiguous slices of 32768 nodes.
Per core:
  projection  t[l,r,b] = sum_c u[l,c,r] * x[b,c]   (per block, all 8 levels)
              done in two passes: level-pairs (0,1) first, then (2,3),
              so the cross-core collective overlaps the second pass
  tree        combine L7-block partials up to coarser blocks
  A2A         exchange levels 0-2 sibling coefficients across cores
              (sender-side 0/1 masks make the combination core-invariant)
  expansion   corr[b,n] = sum_{l,r} u[l,n,r] * t_sib[l,r,b]
u/x are fed as fp8e4m3 (u scaled by USCALE; host divides the returned
correction by USCALE^2). The expansion runs fp8 DoubleRow matmuls that
contract two level-pairs (K=256) per instruction. Host computes diag*x
in fp32 and adds the device-computed correction.
"""

import os
import sys

sys.path.insert(0, "/opt/trn_rl_repo")

import numpy as np
import ml_dtypes

BF16 = ml_dtypes.bfloat16
FP8 = ml_dtypes.float8_e4m3

B = 64
N = 262144
NCORES = 8
M = N // NCORES          # 32768 nodes per core
R = 64
DEPTH = 8
CH = M // 128            # 256 chunks of 128 nodes
NB7 = M // 1024          # 32 L7 blocks (1024 nodes each)
USCALE = 64.0            # u is fed as u*USCALE in fp8 (e4m3 max finite 240)

_cached = {}


def _build_bass():
    import concourse.bacc as bacc
    import concourse.tile as tile
    import concourse.mybir as mybir
    from contextlib import ExitStack

    BF = mybir.dt.bfloat16
    F8 = mybir.dt.float8e4
    F32 = mybir.dt.float32
    ADD = mybir.AluOpType.add
    MULT = mybir.AluOpType.mult

    nc = bacc.Bacc(
        "TRN2",
        target_bir_lowering=False,
        debug=False,
        enable_asserts=False,
        num_devices=NCORES,
    )

    xt_d = nc.dram_tensor("xt", [128, CH, B], F8, kind="ExternalInput").ap()
    # u packed phase-major: [:, ph, k, :] holds level-pairs (2ph, 2ph+1)
    u_d = nc.dram_tensor("u", [128, 2, CH, 256], F8, kind="ExternalInput").ap()
    ut_d = nc.dram_tensor("ut", [4, 128, M], F8, kind="ExternalInput").ap()
    mA_d = nc.dram_tensor("maskA", [128, 8, B], BF, kind="ExternalInput").ap()
    mB_d = nc.dram_tensor("maskB", [64, 8, B], BF, kind="ExternalInput").ap()
    corr_d = nc.dram_tensor("corr", [B, M], F8, kind="ExternalOutput").ap()

    with tile.TileContext(nc) as tc, ExitStack() as ctx:
        const = ctx.enter_context(tc.tile_pool(name="const", bufs=1))
        upool = ctx.enter_context(tc.tile_pool(name="upool", bufs=7))
        pp = ctx.enter_context(tc.tile_pool(name="pp", bufs=1, space="PSUM"))
        accp = ctx.enter_context(tc.tile_pool(name="accp", bufs=1, space="PSUM"))
        accr = ctx.enter_context(tc.tile_pool(name="accr", bufs=2, space="PSUM"))
        treep = ctx.enter_context(tc.tile_pool(name="treep", bufs=1))
        statp = ctx.enter_context(tc.tile_pool(name="statp", bufs=1))
        utp = ctx.enter_context(tc.tile_pool(name="utp", bufs=6))
        ep = ctx.enter_context(tc.tile_pool(name="ep", bufs=4, space="PSUM"))
        yp = ctx.enter_context(tc.tile_pool(name="yp", bufs=3))
        dram = ctx.enter_context(tc.tile_pool(name="dram", bufs=1, space="DRAM"))

        # xt as 4 independent tiles so projection can start after the first
        # quarter lands (whole-tile dependency tracking)
        xt_t = []
        for xq in range(4):
            t_ = const.tile([128, 64, B], F8, tag=f"xt{xq}")
            nc.gpsimd.dma_start(t_[:], xt_d[:, 64 * xq : 64 * (xq + 1), :])
            xt_t.append(t_)

        def xt_chunk(k):
            return xt_t[k // 64][:, k % 64, :]

        mA = const.tile([128, 8, B], BF, tag="mA")
        nc.scalar.dma_start(mA[:], mA_d[:])
        mB = const.tile([64, 8, B], BF, tag="mB")
        nc.scalar.dma_start(mB[:], mB_d[:])

        # ------------- projection with PSUM tree-accumulation -------------
        # Per-level block sums accumulate directly in PSUM across their
        # contributing L7 blocks (start on first MM, stop on last): the DVE
        # tree reduces to a few drains + pair-adds, and the collective's
        # inputs are ready right after the last phase-0 matmul.
        # G[(q, sz)][j]: [128, 64] bf16; rows 0:64 -> level 2q, rows
        # 64:128 -> level 2q+1 of the j-th block of `sz` nodes.
        G = {}

        def tcopy(i, out, in_):
            (nc.vector if i % 2 == 0 else nc.any).tensor_copy(out, in_)

        def tadd(i, out, a, b):
            if i % 2 == 0:
                nc.vector.tensor_tensor(out, a, b, op=ADD)
            else:
                nc.any.tensor_add(out, a, b)

        # long-lived PSUM accumulators: q0 -> one 32768-sum, q1 -> two
        # 16384-halves, q2 -> eight 4096-sums ([128, B] f32 each)
        # hardware clears has_written for the WHOLE 2KB bank on start=True,
        # so every concurrently-open accumulation group gets its own bank:
        # acc0 holds one bank all of phase 0; the q1-half / q2-4096 sums
        # cycle through a 2-bank ring, drained eagerly at their stop.
        acc0 = accp.tile([128, B], F32, tag="acc0", name="acc0")
        acc_cur = None

        for ph, qs in ((0, (0, 1)), (1, (2, 3))):
            for j in range(NB7):
                if j % 4 == 0:
                    u_t2 = upool.tile(
                        [128, 32, 256], F8, tag="u_in", name=f"u_t{ph}_{j}"
                    )
                    nc.sync.dma_start(
                        u_t2[:], u_d[:, ph, 8 * j : 8 * j + 32, :]
                    )
                u_t = u_t2[:, 8 * (j % 4) : 8 * (j % 4) + 8, :]
                for qi, q in enumerate(qs):
                    if q == 0:
                        ps, st, sp = acc0, j == 0, j == NB7 - 1
                    elif q == 1:
                        if j % 16 == 0:
                            acc_cur = accr.tile(
                                [128, B], F32, tag="accr",
                                name=f"acc1_{j // 16}",
                            )
                        ps, st, sp = acc_cur, j % 16 == 0, j % 16 == 15
                    elif q == 2:
                        if j % 4 == 0:
                            acc_cur = accr.tile(
                                [128, B], F32, tag="accr",
                                name=f"acc2_{j // 4}",
                            )
                        ps, st, sp = acc_cur, j % 4 == 0, j % 4 == 3
                    else:
                        ps = pp.tile([128, B], F32, tag="proj", name=f"ps3_{j}")
                        st, sp = True, True
                    for ki in range(8):
                        k = 8 * j + ki
                        nc.tensor.matmul(
                            ps[:],
                            u_t[:, ki, 128 * qi : 128 * qi + 128],
                            xt_chunk(k),
                            start=(st and ki == 0),
                            stop=(sp and ki == 7),
                        )
                    if q == 1 and j % 16 == 15:
                        g = treep.tile([128, B], BF, tag=f"G1_16384_{j // 16}")
                        nc.vector.tensor_copy(g[:], ps[:])
                        G.setdefault((1, 16384), []).append(g)
                    elif q == 2 and j % 4 == 3:
                        g = treep.tile([128, B], BF, tag=f"G2_4096_{j // 4}")
                        tcopy(j // 4, g[:], ps[:])
                        G.setdefault((2, 4096), []).append(g)
                    elif q == 3:
                        g = treep.tile([128, B], BF, tag=f"G3_1024_{j}")
                        tcopy(j, g[:], ps[:])
                        G.setdefault((3, 1024), []).append(g)

            if ph == 0:
                # drain the q0 top and launch the collective immediately
                A = treep.tile([128, B], BF, tag="G0_top")
                nc.vector.tensor_copy(A[:], acc0[:])
                g1h = G[(1, 16384)]
                Bt = treep.tile([64, B], BF, tag="G1_top")
                nc.vector.tensor_tensor(
                    Bt[:], g1h[0][0:64, :], g1h[1][0:64, :], op=ADD
                )
                # ------------- collective (overlaps phase 1) -------------
                # AllGather the raw level-0..2 partials; apply the
                # sibling-selection masks on the receive side (the
                # sibling relation is symmetric, so the same masks work).
                b_in = dram.tile([192, B], BF, tag="b_in")
                b_out = dram.tile(
                    [8, 192, B], BF, tag="b_out", addr_space="Shared"
                )
                nc.scalar.dma_start(b_in[0:128, :], A[:])
                nc.scalar.dma_start(b_in[128:192, :], Bt[:])
                nc.gpsimd.collective_compute(
                    "AllGather",
                    mybir.AluOpType.bypass,
                    replica_groups=[list(range(NCORES))],
                    ins=[b_in.opt()],
                    outs=[b_out.opt()],
                )
                # recvs as two batched SWDGE (gpsimd) DMAs: they complete
                # only after the AllGather, and on an HWDGE queue their
                # pending completions would head-of-line block every later
                # HWDGE DMA sharing their semaphore lane (the ut prefetch)
                recvA = statp.tile([128, 8, B], BF, tag="recvA")
                recvB = statp.tile([64, 8, B], BF, tag="recvB")
                nc.gpsimd.dma_start(
                    recvA[:], b_out[:, 0:128, :].transpose([1, 0, 2])
                )
                nc.gpsimd.dma_start(
                    recvB[:], b_out[:, 128:192, :].transpose([1, 0, 2])
                )
                # masked receive-combine
                mskA = statp.tile([128, 8, B], BF, tag="mskA")
                mskB = statp.tile([64, 8, B], BF, tag="mskB")
                for k in range(8):
                    nc.vector.tensor_tensor(
                        mskA[:, k, :], recvA[:, k, :], mA[:, k, :], op=MULT
                    )
                    nc.vector.tensor_tensor(
                        mskB[:, k, :], recvB[:, k, :], mB[:, k, :], op=MULT
                    )
                tallA = statp.tile([128, B], BF, tag="tallA")
                tallB = statp.tile([64, B], BF, tag="tallB")
                nc.vector.tensor_tensor(
                    tallA[:], mskA[:, 0, :], mskA[:, 1, :], op=ADD
                )
                nc.vector.tensor_tensor(
                    tallB[:], mskB[:, 0, :], mskB[:, 1, :], op=ADD
                )
                for k in range(2, 8):
                    nc.vector.tensor_tensor(
                        tallA[:], tallA[:], mskA[:, k, :], op=ADD
                    )
                    nc.vector.tensor_tensor(
                        tallB[:], tallB[:], mskB[:, k, :], op=ADD
                    )
            else:
                # pair-add q2's 4096 drains into 8192s, q3's 1024s into 2048s
                g4 = G[(2, 4096)]
                G[(2, 8192)] = []
                for m in range(4):
                    g = treep.tile([128, B], BF, tag=f"G2_8192_{m}")
                    tadd(m, g[:], g4[2 * m][:], g4[2 * m + 1][:])
                    G[(2, 8192)].append(g)
                lst = G[(3, 1024)]
                G[(3, 2048)] = []
                for m in range(16):
                    g = treep.tile([128, B], BF, tag=f"G3_2048_{m}")
                    tadd(m + 1, g[:], lst[2 * m][:], lst[2 * m + 1][:])
                    G[(3, 2048)].append(g)

        # prefetch first expansion ut tiles (sync queue: FIFO behind the u
        # loads, so they stream as soon as u is done -- never blocked behind
        # the collective sends/recvs which live on the scalar queue)
        ut_pre = []
        for f in range(2):
            t_ = utp.tile([128, 2, 4096], F8, tag=f"utf{f}", name=f"ut_pre{f}")
            for j in range(2):
                nc.sync.dma_start(t_[:, j, :], ut_d[2 * f + j, :, 0:4096])
            ut_pre.append(t_)

        # ---------------- expansion stationaries (fp8, DoubleRow) --------
        # statf[q-pair-fuse] tiles [128, 2, B]: [:, j, :] holds the
        # stationary of level-pair (2*fuse + j); rows 0:64 = t_sib at the
        # even level of that pair, rows 64:128 = at the odd level.
        statf01 = []
        for m3 in range(2):
            s = statp.tile([128, 2, B], F8, tag=f"sf01_{m3}", name=f"sf01_{m3}")
            nc.vector.tensor_copy(s[:, 0, :], tallA[:])
            nc.vector.tensor_copy(s[0:64, 1, :], tallB[:])
            nc.vector.tensor_copy(
                s[64:128, 1, :], G[(1, 16384)][m3 ^ 1][64:128, :]
            )
            statf01.append(s)
        statf23 = []
        for m7 in range(NB7):
            s = statp.tile([128, 2, B], F8, tag=f"sf23_{m7}", name=f"sf23_{m7}")
            m5 = m7 // 4
            nc.vector.tensor_copy(
                s[0:64, 0, :], G[(2, 8192)][(m5 // 2) ^ 1][0:64, :]
            )
            nc.vector.tensor_copy(
                s[64:128, 0, :], G[(2, 4096)][m5 ^ 1][64:128, :]
            )
            nc.vector.tensor_copy(
                s[0:64, 1, :], G[(3, 2048)][(m7 // 2) ^ 1][0:64, :]
            )
            nc.vector.tensor_copy(
                s[64:128, 1, :], G[(3, 1024)][m7 ^ 1][64:128, :]
            )
            statf23.append(s)

        # ---------------- expansion (DoubleRow fp8) ----------------
        DR = mybir.MatmulPerfMode.DoubleRow
        for gb in range(8):  # 8 blocks of 8 groups x 512 nodes
            if gb == 0:
                ut_t = ut_pre
            else:
                ut_t = [
                    utp.tile(
                        [128, 2, 4096], F8, tag=f"utf{f}", name=f"utf{f}_{gb}"
                    )
                    for f in range(2)
                ]
                for f in range(2):
                    for j in range(2):
                        nc.sync.dma_start(
                            ut_t[f][:, j, :],
                            ut_d[2 * f + j, :, 4096 * gb : 4096 * (gb + 1)],
                        )
            y_sb = yp.tile([B, 4096], F8, tag="y")
            # 4-group sub-batches (matching the 4-bank ep ring): all the
            # statf23 MMs first, then the statf01 MMs -- consecutive MMs
            # reuse/alternate fewer stationaries, amortizing the DR
            # LDWEIGHTS (FWL is off in DR mode, so each load is ~107ns)
            for base in (0, 4):
                epss = []
                for i in range(4):
                    gg = base + i
                    g = 8 * gb + gg
                    eps = ep.tile([B, 512], F32, tag="exp", name=f"eps{g}")
                    sl = slice(512 * gg, 512 * (gg + 1))
                    nc.tensor.matmul(
                        eps[:], statf23[g // 2][:], ut_t[1][:, :, sl],
                        start=True, stop=False, perf_mode=DR,
                    )
                    epss.append((gg, eps, sl))
                for gg, eps, sl in epss:
                    g = 8 * gb + gg
                    nc.tensor.matmul(
                        eps[:], statf01[g // 32][:], ut_t[0][:, :, sl],
                        start=False, stop=True, perf_mode=DR,
                    )
                # split PSUM drains across DVE and ACT (both read PSUM)
                for gg, eps, sl in epss:
                    if gg % 2 == 0:
                        nc.vector.tensor_copy(y_sb[:, sl], eps[:])
                    else:
                        nc.scalar.copy(y_sb[:, sl], eps[:])
            # corr stores on the gpsimd (SWDGE) queue: keeps the sync queue
            # free to stream ut tiles without head-of-line blocking
            nc.gpsimd.dma_start(
                corr_d[:, 4096 * gb : 4096 * (gb + 1)], y_sb[:]
            )

    nc.compile()
    return nc


def _pack_inputs(x, diag, u):
    """Build per-core input maps. x (B,N,1) f32, u (DEPTH,N,R) f32."""
    in_maps = []
    x2 = np.asarray(x).reshape(B, N)
    u3 = np.asarray(u)
    for c in range(NCORES):
        base = c * M
        xsl = x2[:, base : base + M]                      # (B, M)
        usl = u3[:, base : base + M, :] * USCALE          # (8, M, 64)
        xt = np.ascontiguousarray(
            xsl.T.reshape(CH, 128, B).transpose(1, 0, 2)
        ).astype(FP8)                                     # [128, CH, B]
        up = np.ascontiguousarray(
            usl.transpose(1, 0, 2).reshape(M, 512)        # [n, l*64+r]
            .reshape(CH, 128, 2, 256)
            .transpose(1, 2, 0, 3)
        ).astype(FP8)                                     # [128, 2, CH, 256]
        utp = np.ascontiguousarray(
            usl.transpose(0, 2, 1).reshape(4, 128, M)
        ).astype(FP8)                                     # [4, 128, M]
        # masks: mask[d, l] = 1 iff this core c is in the level-l sibling
        # block of destination core d.
        mA = np.zeros((128, 8, B), dtype=BF16)
        mB = np.zeros((64, 8, B), dtype=BF16)
        for d in range(8):
            if (c // 4) == ((d // 4) ^ 1):
                mA[0:64, d, :] = 1.0   # level 0
            if (c // 2) == ((d // 2) ^ 1):
                mA[64:128, d, :] = 1.0  # level 1
            if c == d ^ 1:
                mB[:, d, :] = 1.0       # level 2
        in_maps.append({"xt": xt, "u": up, "ut": utp, "maskA": mA, "maskB": mB})
    return in_maps


last_results = None


def kernel(x, diag, u):
    global last_results
    from concourse.bass_utils import run_bass_kernel_spmd

    if "nc" not in _cached:
        _cached["nc"] = _build_bass()
    nc = _cached["nc"]

    in_maps = _pack_inputs(x, diag, u)
    res = run_bass_kernel_spmd(nc, in_maps, core_ids=list(range(NCORES)))
    last_results = res

    x2 = np.asarray(x, dtype=np.float32).reshape(B, N)
    d2 = np.asarray(diag, dtype=np.float32).reshape(1, N)
    y = d2 * x2
    inv = 1.0 / (USCALE * USCALE)
    for c in range(NCORES):
        y[:, c * M : (c + 1) * M] += res.results[c]["corr"].astype(np.float32) * inv
    return y.reshape(B, N, 1).astype(np.float32)

